# revision 3
# baseline (speedup 1.0000x reference)
"""DTSemNet forward (nn_DTSemNet_54528904790526) on 8 TRN2 NeuronCores.

Math: the reference computes
    x = in_x @ W1.T + b1                       [B, 2047]
    h = [relu(x), relu(-x)]                    [B, 4094]
    z = h @ L.T                                [B, 2048]   (frozen 0/1 leaf routing)
    out[b, a] = max over leaves ell with (ell % 10 == a) of z[b, ell]

L is the complete-binary-tree path matrix: for leaf ell the row is 1 everywhere
except, at each of the 11 path nodes, the half corresponding to the direction
NOT taken. Hence
    z[b, ell] = sum_i |x_i|  -  sum_{path nodes} penalty,
    penalty   = relu(-x_node) when going left, relu(x_node) when going right.
So z = S_abs - cost(leaf) where cost is an 11-level tree DP — this replaces the
dense [B,4094]x[4094,2048] matmul (80% of reference FLOPs) with O(n_leaves)
vector work.

Per-core shard: batch rows (data parallel over 8 cores, 2048 rows each).

The linear1 matmul runs in fp8 e4m3 with perf_mode=DoubleRow: the PE packs two
fp8 weights per cell, virtualizing the contraction to K=256 per pass (2 MACs
per cell per cycle — 2x the fp32r/bf16 column rate). W1 and b1 are pre-scaled
by 32 (power of two, exact) so the ~+/-0.022 weights land mid-range in e4m3;
the activation descales by 1/32. Accuracy: per-element product rel err ~3%
random-walks over K=2048 to sigma(x_i) ~ 0.02; the output is dominated by
S_abs ~ 940 whose error is ~0.9 abs -> ~1e-3 relative, far under the 2e-2
gate (measured 3e-4 rms).

The bias is folded in as an extra ones-row contraction. The DP runs on the
vector engine in bf16 (cost values are O(10) sums of relu's; bf16 absolute
error ~3e-3 => ~3e-6 relative on the output).

Leaf costs are kept in a "split" layout (evens | odds by natural leaf index)
so every DP write is contiguous; the parent interleave is a strided read and
each level is ONE tensor_tensor (parent broadcast via a stride-0 AP dim).
Group-min: leaf ell = 2m+s, ell % 10 = a  <=>  s = a%2, m ≡ a//2 (mod 5);
a TT-min fold 1020->510 per half (multiples of 5 preserve residues) then one
[r=5, j=102]-structured strided reduce per half + a 4-element leftover fixup.
"""
import sys

sys.path.insert(0, "/opt/trn_rl_repo")
from contextlib import ExitStack

import numpy as np
import ml_dtypes

import concourse.bass as bass
import concourse.tile as tile
from concourse import bacc, mybir
from concourse.bass_utils import run_bass_kernel_spmd

# problem shape (hardcoded per contract)
B = 16384
D = 2048
N = 2047          # internal nodes
NP = 2048         # N padded (even/512-multiple free dims)
HEIGHT = 11
NL = 2048         # leaves
OUT = 10
NCORES = 8
BC = B // NCORES  # batch rows per core (2048)
KT = D // 128     # 16 k-tiles
KT2 = KT // 2     # 8 DoubleRow super-k tiles (K=256 each)
BT = BC // 128    # 16 batch tiles per core
CHUNKS = [(0, 512), (512, 1024), (1024, 1536), (1536, 2048)]
WSCALE = 32.0     # power-of-two pre-scale for fp8 weights/bias

f32 = mybir.dt.float32
f32r = mybir.dt.float32r
bf16 = mybir.dt.bfloat16
fp8 = mybir.dt.float8e4
np_fp8 = ml_dtypes.float8_e4m3
ADD = mybir.AluOpType.add
MIN = mybir.AluOpType.min
SUB = mybir.AluOpType.subtract
MULT = mybir.AluOpType.mult
RELU = mybir.ActivationFunctionType.Relu
AXX = mybir.AxisListType.X
DR = mybir.MatmulPerfMode.DoubleRow


def build_kernel(bt=BT, reps=1, loop_reps=None, mode="full"):
    """bt: number of batch tiles (128 rows each) this kernel processes.
    reps: python-unrolled repeats of the whole per-tile pipeline.
    loop_reps: device-side For_i repeats (for timing probes).
    mode: "full" | "nodp" (skip tree DP/mins) | "mmonly" (matmuls only)
          | "dponly" (skip matmuls)."""
    nc = bacc.Bacc("TRN2")
    # in_x.T shard in fp8, pre-blocked host-side as [bt][128 p][KT k][128 m]
    # with p = contraction row % 128, so each SBUF partition reads one
    # contiguous 2KB run per batch tile.
    xt = nc.dram_tensor("xt", [bt * 128, KT * 128], fp8, kind="ExternalInput")
    # W1.T * 32, fp8, DoubleRow pair layout: block j is [128 p, 2, NP] flat
    # as [128, 2*NP] with pair element u = k-tile 2j+u, row p = k % 128.
    wdr = nc.dram_tensor("wdr", [KT2 * 128, 2 * NP], fp8, kind="ExternalInput")
    ones = nc.dram_tensor("ones", [1, 128], fp8, kind="ExternalInput")
    # b1 * 32 broadcast to 128 partitions (only partitions 0/32/64/96 read)
    wtb4 = nc.dram_tensor("wtb4", [128, NP], fp8, kind="ExternalInput")
    out = nc.dram_tensor("out", [bt * 128, OUT], f32, kind="ExternalOutput")

    with tile.TileContext(nc) as tc, ExitStack() as ctx:
        wt_pool = ctx.enter_context(tc.tile_pool(name="wt", bufs=1))
        xt_pool = ctx.enter_context(tc.tile_pool(name="xt", bufs=3))
        ps_pool = ctx.enter_context(tc.tile_pool(name="ps", bufs=2, space="PSUM"))
        pen_pool = ctx.enter_context(tc.tile_pool(name="pen", bufs=2))
        dp_pool = ctx.enter_context(tc.tile_pool(name="dp", bufs=3))
        sm_pool = ctx.enter_context(tc.tile_pool(name="sm", bufs=4))

        # resident weights: 8 DoubleRow super-k tiles + bias row
        wts = []
        for j in range(KT2):
            wj = wt_pool.tile([128, 2 * NP], fp8, tag=f"wt{j}")
            nc.sync.dma_start(wj[:], wdr[j * 128:(j + 1) * 128, :])
            wts.append(wj)
        # b1 and ones rows replicated at partitions 0/32/64/96 so the four
        # K=1 bias matmuls can run as concurrent PE row-group tiles
        wtb_t = wt_pool.tile([128, NP], fp8, tag="wtb")
        nc.sync.dma_start(wtb_t[:], wtb4[:, :])
        ones_t = wt_pool.tile([128, 128], fp8, tag="ones")
        for rg in range(4):
            nc.sync.dma_start(ones_t[32 * rg:32 * rg + 1, :], ones[0:1, :])

        def body():
            for t in range(bt):
                c_lo = t * 128
                c_hi = (t + 1) * 128
                xt_t = xt_pool.tile([128, KT * 128], fp8, tag="xt")
                nc.sync.dma_start(xt_t[:], xt[c_lo:c_hi, :])

                pen = pen_pool.tile([128, 2 * NP], bf16, tag="pen")
                sacc = sm_pool.tile([128, 2 * len(CHUNKS)], f32, tag="sacc")

                if mode == "dponly":
                    # fill pen/sacc cheaply so the DP chain is isolated
                    nc.vector.memset(pen[:], 0.5)
                    nc.vector.memset(sacc[:], 1.0)
                else:
                    # j-outer order: 4 consecutive matmuls share the
                    # stationary xt pair-block
                    pss = [ps_pool.tile([128, 512], f32, tag=f"ps{ci}",
                                        name=f"ps{ci}")
                           for ci in range(len(CHUNKS))]
                    for j in range(KT2):
                        lhsT = xt_t[:, j * 256:(j + 1) * 256].rearrange(
                            "p (u m) -> p u m", u=2)
                        for ci, (c0, c1) in enumerate(CHUNKS):
                            rhs = wts[j][:].rearrange(
                                "p (u n) -> p u n", u=2)[:, :, c0:c1]
                            nc.tensor.matmul(
                                pss[ci][:, 0:c1 - c0], lhsT, rhs,
                                start=(j == 0), stop=False, perf_mode=DR,
                            )
                    for ci, (c0, c1) in enumerate(CHUNKS):
                        w = c1 - c0
                        bp = 32 * ci
                        nc.tensor.matmul(
                            pss[ci][:, 0:w],
                            ones_t[bp:bp + 1, 0:128],
                            wtb_t[bp:bp + 1, c0:c1],
                            start=False, stop=True,
                            tile_position=(bp, 0),
                        )
                    if mode == "mmonly":
                        outsb = sm_pool.tile([128, OUT], f32, tag="outsb")
                        nc.scalar.copy(outsb[:], pss[0][:, 0:OUT])
                        nc.sync.dma_start(out[c_lo:c_hi, :], outsb[:])
                        continue
                    for ci, (c0, c1) in enumerate(CHUNKS):
                        w = c1 - c0
                        ps = pss[ci]
                        # pr = relu(x) -> pen[NP + c], with running row-sum
                        # (PSUM holds 32*x; the activation descales)
                        nc.scalar.activation(
                            pen[:, NP + c0:NP + c1], ps[:, 0:w], RELU,
                            scale=1.0 / WSCALE,
                            accum_out=sacc[:, 2 * ci:2 * ci + 1],
                        )
                        # pl = relu(-x) -> pen[c]
                        nc.scalar.activation(
                            pen[:, c0:c1], ps[:, 0:w], RELU,
                            scale=-1.0 / WSCALE,
                            accum_out=sacc[:, 2 * ci + 1:2 * ci + 2],
                        )

                sabs = sm_pool.tile([128, 1], f32, tag="sabs")
                nc.vector.tensor_reduce(sabs[:], sacc[:], axis=AXX, op=ADD)
                if mode == "nodp":
                    outsb = sm_pool.tile([128, OUT], f32, tag="outsb")
                    nc.scalar.copy(outsb[:, 0:8], sacc[:])
                    nc.sync.dma_start(out[c_lo:c_hi, 0:8], outsb[:, 0:8])
                    continue

                # ---- tree DP over 11 levels, split (evens|odds) layout ----
                # One TT per level: out[s, j, u] = par[j, u] + pen[s][n0 + 2j+u]
                # (s = 0 left / 1 right half; parent broadcast via stride-0 dim)
                # level-1 costs are pen[0] (left child) and pen[NP] (right)
                par = pen[:, 0:2 * NP:NP].rearrange("p (j u) -> p j u", u=2)
                pen2 = pen.rearrange("p (s c) -> p s c", s=2)
                lvl = None
                for d in range(1, HEIGHT):
                    w = 1 << d          # number of level-d nodes = parents
                    n0 = w - 1          # first node index of level d
                    nxt = dp_pool.tile([128, 2 * w], bf16, tag=f"lvl{d + 1}")
                    out4 = nxt[:].rearrange("p (s j u) -> p s j u", s=2, u=2)
                    pen4 = pen2[:, :, n0:n0 + w].rearrange(
                        "p s (j u) -> p s j u", u=2)
                    par4 = par.rearrange("p (x j) u -> p x j u", x=1)
                    par4 = par4.broadcast_to([128, 2, w // 2, 2])
                    nc.vector.tensor_tensor(out4, par4, pen4, op=ADD)
                    lvl = nxt
                    par = nxt[:].rearrange("p (u j) -> p j u", u=2)

                # lvl holds leaf costs [128, 2048]: evens | odds halves.
                # group a=2r+s: min over positions m ≡ r (mod 5) of half s.
                # group-min pre-folds (offsets multiple of 5 keep residues):
                # TT-min folds 1020 -> 510 -> 255 per half, then one strided
                # reduce per half over 255 + a 4-elem leftover fixup on DVE.
                half = NL // 2
                lvl2 = lvl[:].rearrange("p (s c) -> p s c", s=2)
                fold = dp_pool.tile([128, 2 * 512], bf16, tag="fold")
                fold2 = fold[:].rearrange("p (s c) -> p s c", s=2)
                nc.vector.tensor_tensor(
                    fold2[:, :, 0:510], lvl2[:, :, 0:510],
                    lvl2[:, :, 510:1020], op=MIN,
                )
                nc.vector.tensor_tensor(
                    fold2[:, :, 0:255], fold2[:, :, 0:255],
                    fold2[:, :, 255:510], op=MIN,
                )
                tmp = sm_pool.tile([128, 2 * 5], bf16, tag="mins")
                for s in range(2):
                    src = fold2[:, s, 0:255].rearrange("p (j r) -> p r j", r=5)
                    nc.vector.tensor_reduce(
                        tmp[:, s * 5:s * 5 + 5], src, axis=AXX, op=MIN,
                    )
                    nc.vector.tensor_tensor(
                        tmp[:, s * 5:s * 5 + 4], tmp[:, s * 5:s * 5 + 4],
                        lvl[:, s * half + 1020:s * half + 1024], op=MIN,
                    )

                outsb = sm_pool.tile([128, OUT], f32, tag="outsb")
                # out[:, 2r+s] = (tmp[s*5+r] - S_abs) * -1 = S_abs - min
                out_perm = outsb[:].rearrange("p (r s) -> p s r", s=2)
                nc.vector.tensor_scalar(
                    out_perm, tmp[:].rearrange("p (s r) -> p s r", s=2),
                    sabs[:, 0:1], -1.0, op0=SUB, op1=MULT,
                )
                nc.sync.dma_start(out[c_lo:c_hi, :], outsb[:])

        if loop_reps is not None:
            with tc.For_i(0, loop_reps):
                body()
        else:
            for _ in range(reps):
                body()

    nc.finalize()
    return nc


_NC_CACHE = {}


def _get_nc():
    key = (BT, 1)
    if key not in _NC_CACHE:
        _NC_CACHE[key] = build_kernel()
    return _NC_CACHE[key]


def marshal_xt(in_x_shard):
    """[BC, D] rows -> [BT*128, KT*128] fp8 blocked so that SBUF partition p
    of batch-tile t reads contiguously: out[t*128+p, k*128+m] =
    in_x_shard[t*128+m, k*128+p]."""
    a = np.asarray(in_x_shard, np.float32).reshape(BT, 128, KT, 128)
    return np.ascontiguousarray(
        a.transpose(0, 3, 2, 1).reshape(BT * 128, KT * 128)).astype(np_fp8)


def marshal_wdr(W1, b1):
    """W1 [N, D], b1 [N] -> (wdr [KT2*128, 2*NP], wtb4 [128, NP]) fp8,
    scaled by WSCALE. wdr block j pairs k-tiles 2j / 2j+1 along dim u."""
    wt_full = np.zeros((D, NP), np.float32)
    wt_full[:, :N] = W1.T * WSCALE
    wdr = np.ascontiguousarray(
        wt_full.reshape(KT2, 2, 128, NP).transpose(0, 2, 1, 3)
        .reshape(KT2 * 128, 2 * NP)).astype(np_fp8)
    bias = np.zeros((NP,), np.float32)
    bias[:N] = b1 * WSCALE
    wtb4 = np.ascontiguousarray(
        np.broadcast_to(bias, (128, NP))).astype(np_fp8)
    return wdr, wtb4


def default_in_map(seed=0):
    """One core's input map with random data (for timing probes only)."""
    rng = np.random.default_rng(seed)
    in_x = rng.standard_normal((BC, D), np.float32)
    W1 = (rng.standard_normal((N, D), np.float32) * 0.0128).astype(np.float32)
    b1 = (rng.standard_normal((N,), np.float32) * 0.0128).astype(np.float32)
    wdr, wtb4 = marshal_wdr(W1, b1)
    return {"xt": marshal_xt(in_x), "wdr": wdr,
            "ones": np.ones((1, 128), np_fp8), "wtb4": wtb4}


def kernel(in_x, W1, b1, L, A):
    in_x = np.asarray(in_x, np.float32)
    W1 = np.asarray(W1, np.float32)
    b1 = np.asarray(b1, np.float32)
    wdr, wtb4 = marshal_wdr(W1, b1)
    ones = np.ones((1, 128), np_fp8)
    in_maps = [
        {"xt": marshal_xt(in_x[c * BC:(c + 1) * BC]), "wdr": wdr,
         "ones": ones, "wtb4": wtb4}
        for c in range(NCORES)
    ]
    nc = _get_nc()
    res = run_bass_kernel_spmd(nc, in_maps, core_ids=list(range(NCORES)))
    return np.concatenate([res.results[c]["out"] for c in range(NCORES)], axis=0)


# revision 4
# speedup vs baseline: 1.2656x; 1.2656x over previous
"""DTSemNet forward (nn_DTSemNet_54528904790526) on 8 TRN2 NeuronCores.

Math: the reference computes
    x = in_x @ W1.T + b1                       [B, 2047]
    h = [relu(x), relu(-x)]                    [B, 4094]
    z = h @ L.T                                [B, 2048]   (frozen 0/1 leaf routing)
    out[b, a] = max over leaves ell with (ell % 10 == a) of z[b, ell]

L is the complete-binary-tree path matrix, so
    z[b, ell] = sum_i |x_i|  -  sum_{path nodes} penalty,
    penalty   = relu(-x_node) going left, relu(x_node) going right.
The dense [B,4094]x[4094,2048] matmul becomes an 11-level tree DP on the
vector engine; the remaining linear1 matmul runs in fp8 e4m3 with
perf_mode=DoubleRow (two fp8 weights per PE cell -> K=256 per pass, 2x the
fp32r/bf16 column rate). W1/b1 are pre-scaled by 32 (exact power of two);
the activation descales by 1/32.

Layout tricks (all host-side, free):
 - weight columns are stored by SLOT, not node index: slot 0 is a "magic"
   column holding sum_n W1[n,:] (unscaled) so psum[0] = sum_n x_n per row,
   giving S_abs = 2*sum(relu(x)) - sum(x) with a single Act accumulator;
   level-d nodes occupy slots [2^d, 2^(d+1)) (4B-aligned starts).
 - DP levels 1-6 keep natural within-level order (strided parent re-read,
   DVE 1x, tiny). Levels 7-10 switch to contiguous append-at-top form:
   C_{d+1}[s*2^d + i] = C_d[i] + pen[s][slot 2^d + i], all operands
   stride-1 so the DVE runs 2x_1P. The weight columns for those levels are
   permuted (eta) so slot order matches the recursion; the resulting leaf
   order is leaf(q) = 16*eta7(P) + rev4(q>>7).
 - group-min (leaf % 10): position q = B*128 + s2*64 + m has group
   (2*(m%5) + alpha(B,s2)) % 10, so: fold m 64->30->10->5 per (B,s2) block
   (mod-5-preserving offsets), then merge the 32 (B,s2) pentads class-wise
   (host-planned TT batches), land the 10 survivors in slots ordered by
   rotation offset, duplicate, and one strided tensor_reduce yields all 10
   group-mins. out[a] = S_abs - min.
"""
import sys

sys.path.insert(0, "/opt/trn_rl_repo")
from contextlib import ExitStack

import numpy as np
import ml_dtypes

import concourse.bass as bass
import concourse.tile as tile
from concourse import bacc, mybir
from concourse.bass_utils import run_bass_kernel_spmd
from bass_rust import VecI64Pair

# problem shape (hardcoded per contract)
B = 16384
D = 2048
N = 2047          # internal nodes
NP = 2048         # slots (slot 0 = magic sum column, slots 1..2047 = nodes)
HEIGHT = 11
OUT = 10
NCORES = 8
BC = B // NCORES  # batch rows per core (2048)
KT = D // 128     # 16 k-tiles
KT2 = KT // 2     # 8 DoubleRow super-k tiles (K=256 each)
BT = BC // 128    # 16 batch tiles per core
NCH = 4           # 512-wide psum chunks
WSCALE = 32.0     # power-of-two pre-scale for fp8 weights/bias

f32 = mybir.dt.float32
bf16 = mybir.dt.bfloat16
fp8 = mybir.dt.float8e4
np_fp8 = ml_dtypes.float8_e4m3
ADD = mybir.AluOpType.add
MIN = mybir.AluOpType.min
SUB = mybir.AluOpType.subtract
MULT = mybir.AluOpType.mult
RELU = mybir.ActivationFunctionType.Relu
AXX = mybir.AxisListType.X
DR = mybir.MatmulPerfMode.DoubleRow

# ---------------------------------------------------------------------------
# host-side plan: eta permutations, slot map, pentad-tail merge schedule
# ---------------------------------------------------------------------------


def _build_plan():
    etas = {7: 2 * (np.arange(128) % 64) + (np.arange(128) // 64)}
    for d in (7, 8, 9):
        etas[d + 1] = np.concatenate([2 * etas[d], 2 * etas[d] + 1])
    leaf_of_q = np.concatenate([2 * etas[10], 2 * etas[10] + 1])
    a_of_q = leaf_of_q % 10
    A = a_of_q.reshape(16, 2, 64)
    assert (A == A[:, :, np.arange(64) % 5]).all()
    alpha = A[:, :, 0].copy()
    for r in range(5):
        assert (A[:, :, r] == (alpha + 2 * r) % 10).all()
    assert (alpha[:8] % 2 == 0).all() and (alpha[8:] % 2 == 1).all()
    assert ((alpha[8:] - alpha[:8]) % 10 == 1).all()

    node_at_slot = np.full(NP, -1, np.int64)
    for d in range(0, 7):
        w = 1 << d
        node_at_slot[w:2 * w] = (w - 1) + np.arange(w)
    for d in range(7, 11):
        w = 1 << d
        node_at_slot[w:2 * w] = (w - 1) + etas[d]
    assert sorted(node_at_slot[1:].tolist()) == list(range(N))

    # pentad tail (decad positions pos = B*20 + s2*10 in the f2 tile, even
    # parity; odd parity is the same structure at +160 / H +64).
    # classes by alpha: 0:[0,50,100,150] 2:[60,80,130] 4:[30,40,140]
    #                   6:[10,110,120]   8:[20,70,90]
    cls = {}
    for Bb in range(8):
        for s2 in range(2):
            cls.setdefault(int(alpha[Bb, s2]), []).append(Bb * 20 + s2 * 10)
    assert {k: sorted(v) for k, v in cls.items()} == {
        0: [0, 50, 100, 150], 2: [60, 80, 130], 4: [30, 40, 140],
        6: [10, 110, 120], 8: [20, 70, 90]}
    # round 1: in-place merges dst <- min(dst, dst+delta)
    round1 = [(50, [0, 100]), (20, [60]), (10, [30]), (100, [10]), (50, [20])]
    # round 2: final merges, outputs land in H at slot t = (-alpha/2) % 5,
    # H position par*64 + t*11.  alpha->t: 0->0, 2->4, 4->3, 6->2, 8->1.
    round2 = [(100, [0], [0]), (110, [10, 30], [22, 33]),
              (70, [20, 60], [11, 44])]
    return node_at_slot, round1, round2


NODE_AT_SLOT, TAIL_R1, TAIL_R2 = _build_plan()


def _hack_ap(base_ap, dims, extra_offset):
    """Arbitrary strided free-dim AP on the same tensor/partitions."""
    c = base_ap.copy()
    part = base_ap.ap.to_list()[0]
    c.ap = VecI64Pair([part] + [[s, n] for (s, n) in dims])
    c.offset = base_ap.offset + extra_offset
    return c


# ---------------------------------------------------------------------------
# kernel
# ---------------------------------------------------------------------------


def build_kernel(bt=BT, reps=1, loop_reps=None, mode="full"):
    """bt: batch tiles (128 rows each). loop_reps: device-side For_i repeats.
    mode: "full" | "nodp" | "mmonly" | "dponly"."""
    nc = bacc.Bacc("TRN2")
    xt = nc.dram_tensor("xt", [bt * 128, KT * 128], fp8, kind="ExternalInput")
    wdr = nc.dram_tensor("wdr", [KT2 * 128, 2 * NP], fp8, kind="ExternalInput")
    ones = nc.dram_tensor("ones", [1, 128], fp8, kind="ExternalInput")
    wtb4 = nc.dram_tensor("wtb4", [128, NP], fp8, kind="ExternalInput")
    out = nc.dram_tensor("out", [bt * 128, OUT], f32, kind="ExternalOutput")

    with tile.TileContext(nc) as tc, ExitStack() as ctx:
        wt_pool = ctx.enter_context(tc.tile_pool(name="wt", bufs=1))
        xt_pool = ctx.enter_context(tc.tile_pool(name="xt", bufs=3))
        ps_pool = ctx.enter_context(tc.tile_pool(name="ps", bufs=2, space="PSUM"))
        pen_pool = ctx.enter_context(tc.tile_pool(name="pen", bufs=2))
        dp_pool = ctx.enter_context(tc.tile_pool(name="dp", bufs=3))
        sm_pool = ctx.enter_context(tc.tile_pool(name="sm", bufs=4))

        wts = []
        for j in range(KT2):
            wj = wt_pool.tile([128, 2 * NP], fp8, tag=f"wt{j}")
            nc.sync.dma_start(wj[:], wdr[j * 128:(j + 1) * 128, :])
            wts.append(wj)
        wtb_t = wt_pool.tile([128, NP], fp8, tag="wtb")
        nc.sync.dma_start(wtb_t[:], wtb4[:, :])
        ones_t = wt_pool.tile([128, 128], fp8, tag="ones")
        for rg in range(4):
            nc.sync.dma_start(ones_t[32 * rg:32 * rg + 1, :], ones[0:1, :])

        def body():
            for t in range(bt):
                c_lo = t * 128
                c_hi = (t + 1) * 128
                xt_t = xt_pool.tile([128, KT * 128], fp8, tag="xt")
                nc.sync.dma_start(xt_t[:], xt[c_lo:c_hi, :])

                pen = pen_pool.tile([128, 2 * NP], bf16, tag="pen")
                spr = sm_pool.tile([128, 1], f32, tag="spr")
                xsum = sm_pool.tile([128, 1], f32, tag="xsum")

                if mode == "dponly":
                    nc.vector.memset(pen[:], 0.5)
                    nc.vector.memset(spr[:], 500.0)
                    nc.vector.memset(xsum[:], 1.0)
                else:
                    ps = ps_pool.tile([128, NP], f32, tag="ps", name="ps")
                    for j in range(KT2):
                        lhsT = xt_t[:, j * 256:(j + 1) * 256].rearrange(
                            "p (u m) -> p u m", u=2)
                        for ci in range(NCH):
                            c0 = 512 * ci
                            rhs = wts[j][:].rearrange(
                                "p (u n) -> p u n", u=2)[:, :, c0:c0 + 512]
                            nc.tensor.matmul(
                                ps[:, c0:c0 + 512], lhsT, rhs,
                                start=(j == 0), stop=False, perf_mode=DR,
                            )
                    for ci in range(NCH):
                        c0 = 512 * ci
                        bp = 32 * ci
                        nc.tensor.matmul(
                            ps[:, c0:c0 + 512],
                            ones_t[bp:bp + 1, 0:128],
                            wtb_t[bp:bp + 1, c0:c0 + 512],
                            start=False, stop=True,
                            tile_position=(bp, 0),
                        )
                    if mode == "mmonly":
                        outsb = sm_pool.tile([128, OUT], f32, tag="outsb")
                        nc.scalar.copy(outsb[:], ps[:, 0:OUT])
                        nc.sync.dma_start(out[c_lo:c_hi, :], outsb[:])
                        continue
                    # pr = relu(x) over node slots, with row-sum accumulator
                    nc.scalar.activation(
                        pen[:, NP + 1:2 * NP], ps[:, 1:NP], RELU,
                        scale=1.0 / WSCALE, accum_out=spr[:],
                    )
                    # pl = relu(-x)
                    nc.scalar.activation(
                        pen[:, 1:NP], ps[:, 1:NP], RELU, scale=-1.0 / WSCALE,
                    )
                    # magic column: psum[0] = sum_n x_n (unscaled)
                    nc.scalar.copy(xsum[:], ps[:, 0:1])

                # S_abs = 2*sum(relu(x)) - sum(x)
                sabs = sm_pool.tile([128, 1], f32, tag="sabs")
                nc.vector.scalar_tensor_tensor(
                    sabs[:], spr[:], 2.0, xsum[:], op0=MULT, op1=SUB)
                if mode == "nodp":
                    outsb = sm_pool.tile([128, OUT], f32, tag="outsb")
                    nc.scalar.copy(outsb[:, 0:1], sabs[:])
                    nc.sync.dma_start(out[c_lo:c_hi, 0:1], outsb[:, 0:1])
                    continue

                # ---- tree DP ----
                pen2 = pen.rearrange("p (s c) -> p s c", s=2)
                # levels 1..6: natural within-level order, strided parent
                par = pen[:, 1:2 * NP:NP].rearrange("p (j u) -> p j u", u=2)
                lvl = None
                for d in range(1, 7):
                    w = 1 << d
                    nxt = dp_pool.tile([128, 2 * w], bf16, tag=f"lvl{d + 1}")
                    out4 = nxt[:].rearrange("p (s j u) -> p s j u", s=2, u=2)
                    pen4 = pen2[:, :, w:2 * w].rearrange(
                        "p s (j u) -> p s j u", u=2)
                    par4 = par.rearrange("p (x j) u -> p x j u", x=1)
                    par4 = par4.broadcast_to([128, 2, w // 2, 2])
                    nc.vector.tensor_tensor(out4, par4, pen4, op=ADD)
                    lvl = nxt
                    par = nxt[:].rearrange("p (u j) -> p j u", u=2)
                # levels 7..10: append-at-top, all operands stride-1 (DVE 2x)
                for d in range(7, 11):
                    w = 1 << d
                    nxt = dp_pool.tile([128, 2 * w], bf16, tag=f"lvl{d + 1}")
                    out3 = nxt[:].rearrange("p (s i) -> p s i", s=2)
                    par3 = lvl[:].rearrange(
                        "p (x i) -> p x i", x=1).broadcast_to([128, 2, w])
                    pen3 = pen2[:, :, w:2 * w]
                    nc.vector.tensor_tensor(out3, par3, pen3, op=ADD)
                    lvl = nxt

                # ---- group-min ----
                # lvl [p, 2048] viewed [B(16)*128, s2(2)*64, m(64)]
                lvl4 = lvl[:].rearrange("p (b s m) -> p b s m", b=16, s=2)
                f1 = dp_pool.tile([128, 16 * 2 * 30], bf16, tag="f1")
                f1v = f1[:].rearrange("p (b s m) -> p b s m", b=16, s=2)
                nc.vector.tensor_tensor(
                    f1v, lvl4[:, :, :, 0:30], lvl4[:, :, :, 30:60], op=MIN)
                nc.vector.tensor_tensor(
                    f1v[:, :, :, 0:4], f1v[:, :, :, 0:4],
                    lvl4[:, :, :, 60:64], op=MIN)
                f2 = dp_pool.tile([128, 16 * 2 * 10], bf16, tag="f2")
                f2v = f2[:].rearrange("p (b s m) -> p b s m", b=16, s=2)
                nc.vector.tensor_tensor(
                    f2v, f1v[:, :, :, 0:10], f1v[:, :, :, 10:20], op=MIN)
                nc.vector.tensor_tensor(
                    f2v, f2v, f1v[:, :, :, 20:30], op=MIN)
                nc.vector.tensor_tensor(
                    f2v[:, :, :, 0:5], f2v[:, :, :, 0:5],
                    f2v[:, :, :, 5:10], op=MIN)

                # pentad tail: class merges (even parity; odd at +160)
                f2b = f2[:, 0:1]
                for delta, in0s in TAIL_R1:
                    dims = [(160, 2)]
                    if len(in0s) == 2:
                        dims.append((in0s[1] - in0s[0], 2))
                    dims.append((1, 5))
                    a_dst = _hack_ap(f2b, dims, in0s[0])
                    a_src = _hack_ap(f2b, dims, in0s[0] + delta)
                    nc.vector.tensor_tensor(a_dst, a_dst, a_src, op=MIN)
                H = dp_pool.tile([128, 128], bf16, tag="H")
                Hb = H[:, 0:1]
                for delta, in0s, outs in TAIL_R2:
                    dims_in = [(160, 2)]
                    dims_out = [(64, 2)]
                    if len(in0s) == 2:
                        dims_in.append((in0s[1] - in0s[0], 2))
                        dims_out.append((outs[1] - outs[0], 2))
                    dims_in.append((1, 5))
                    dims_out.append((1, 5))
                    a0 = _hack_ap(f2b, dims_in, in0s[0])
                    a1 = _hack_ap(f2b, dims_in, in0s[0] + delta)
                    ao = _hack_ap(Hb, dims_out, outs[0])
                    nc.vector.tensor_tensor(ao, a0, a1, op=MIN)
                # duplicate pentads: H[.. t*11+5+c] = H[.. t*11+c]
                dsrc = _hack_ap(Hb, [(64, 2), (11, 5), (1, 5)], 0)
                ddst = _hack_ap(Hb, [(64, 2), (11, 5), (1, 5)], 5)
                nc.vector.tensor_copy(ddst, dsrc)
                # rotated strided reduce: tmp[par, j] = min_t H[par*64+t*12+j]
                tmp = sm_pool.tile([128, OUT], bf16, tag="mins")
                rsrc = _hack_ap(Hb, [(64, 2), (1, 5), (12, 5)], 0)
                tmpv = tmp[:].rearrange("p (s j) -> p s j", s=2)
                nc.vector.tensor_reduce(tmpv, rsrc, axis=AXX, op=MIN)

                outsb = sm_pool.tile([128, OUT], f32, tag="outsb")
                # out[:, 2j+par] = sabs - tmp[par, j]
                out_perm = outsb[:].rearrange("p (j s) -> p s j", s=2)
                nc.vector.tensor_scalar(
                    out_perm, tmpv, sabs[:, 0:1], -1.0, op0=SUB, op1=MULT)
                nc.sync.dma_start(out[c_lo:c_hi, :], outsb[:])

        if loop_reps is not None:
            with tc.For_i(0, loop_reps):
                body()
        else:
            for _ in range(reps):
                body()

    nc.finalize()
    return nc


_NC_CACHE = {}


def _get_nc():
    key = (BT, 1)
    if key not in _NC_CACHE:
        _NC_CACHE[key] = build_kernel()
    return _NC_CACHE[key]


# ---------------------------------------------------------------------------
# host marshaling
# ---------------------------------------------------------------------------


def marshal_xt(in_x_shard):
    """[BC, D] rows -> [BT*128, KT*128] fp8 blocked: out[t*128+p, k*128+m] =
    in_x_shard[t*128+m, k*128+p]."""
    a = np.asarray(in_x_shard, np.float32).reshape(BT, 128, KT, 128)
    return np.ascontiguousarray(
        a.transpose(0, 3, 2, 1).reshape(BT * 128, KT * 128)).astype(np_fp8)


def marshal_wdr(W1, b1):
    """W1 [N, D], b1 [N] -> (wdr [KT2*128, 2*NP], wtb4 [128, NP]) fp8.
    Slot layout: col 0 = sum of W1 rows (unscaled); node slots * WSCALE.
    DR pair layout: block j pairs k-tiles 2j/2j+1 along dim u."""
    wt_full = np.empty((D, NP), np.float32)
    wt_full[:, 0] = W1.sum(axis=0)
    wt_full[:, 1:] = W1.T[:, NODE_AT_SLOT[1:]] * WSCALE
    wdr = np.ascontiguousarray(
        wt_full.reshape(KT2, 2, 128, NP).transpose(0, 2, 1, 3)
        .reshape(KT2 * 128, 2 * NP)).astype(np_fp8)
    bias = np.empty((NP,), np.float32)
    bias[0] = b1.sum()
    bias[1:] = b1[NODE_AT_SLOT[1:]] * WSCALE
    wtb4 = np.ascontiguousarray(
        np.broadcast_to(bias, (128, NP))).astype(np_fp8)
    return wdr, wtb4


def default_in_map(seed=0):
    """One core's input map with random data (timing probes only)."""
    rng = np.random.default_rng(seed)
    in_x = rng.standard_normal((BC, D), np.float32)
    W1 = (rng.standard_normal((N, D), np.float32) * 0.0128).astype(np.float32)
    b1 = (rng.standard_normal((N,), np.float32) * 0.0128).astype(np.float32)
    wdr, wtb4 = marshal_wdr(W1, b1)
    return {"xt": marshal_xt(in_x), "wdr": wdr,
            "ones": np.ones((1, 128), np_fp8), "wtb4": wtb4}


def kernel(in_x, W1, b1, L, A):
    in_x = np.asarray(in_x, np.float32)
    W1 = np.asarray(W1, np.float32)
    b1 = np.asarray(b1, np.float32)
    wdr, wtb4 = marshal_wdr(W1, b1)
    ones = np.ones((1, 128), np_fp8)
    in_maps = [
        {"xt": marshal_xt(in_x[c * BC:(c + 1) * BC]), "wdr": wdr,
         "ones": ones, "wtb4": wtb4}
        for c in range(NCORES)
    ]
    nc = _get_nc()
    res = run_bass_kernel_spmd(nc, in_maps, core_ids=list(range(NCORES)))
    return np.concatenate([res.results[c]["out"] for c in range(NCORES)], axis=0)


# ---------------------------------------------------------------------------
# numpy end-to-end simulation of the device pipeline (index-algebra check)
# ---------------------------------------------------------------------------


def selfcheck(seed=1, nb=5):
    rng = np.random.default_rng(seed)
    x = rng.standard_normal((nb, N))
    pen = np.zeros((nb, 2, NP))
    nodes = NODE_AT_SLOT[1:]
    pen[:, 0, 1:] = np.maximum(-x[:, nodes], 0)
    pen[:, 1, 1:] = np.maximum(x[:, nodes], 0)
    # DP levels 1..6 (natural, strided par)
    par = np.stack([pen[:, 0, 1], pen[:, 1, 1]], axis=1).reshape(nb, 1, 2)
    for d in range(1, 7):
        w = 1 << d
        pen4 = pen[:, :, w:2 * w].reshape(nb, 2, w // 2, 2)
        out4 = par.reshape(nb, 1, w // 2, 2) + pen4
        nxt = out4.reshape(nb, 2 * w)
        par = np.stack([nxt[:, :w], nxt[:, w:]], axis=2)  # [nb, j, u]
        lvl = nxt
    # levels 7..10 (append-at-top)
    for d in range(7, 11):
        w = 1 << d
        lvl = np.concatenate(
            [lvl + pen[:, 0, w:2 * w], lvl + pen[:, 1, w:2 * w]], axis=1)
    # group-min
    L4 = lvl.reshape(nb, 16, 2, 64)
    f1 = np.minimum(L4[..., 0:30], L4[..., 30:60])
    f1[..., 0:4] = np.minimum(f1[..., 0:4], L4[..., 60:64])
    f2 = np.minimum(f1[..., 0:10], f1[..., 10:20])
    f2 = np.minimum(f2, f1[..., 20:30])
    f2[..., 0:5] = np.minimum(f2[..., 0:5], f2[..., 5:10])
    G = f2.reshape(nb, 320)
    for delta, in0s in TAIL_R1:
        for b0 in in0s:
            for po in (0, 160):
                s = b0 + po
                G[:, s:s + 5] = np.minimum(G[:, s:s + 5],
                                           G[:, s + delta:s + delta + 5])
    H = np.full((nb, 128), np.inf)
    for delta, in0s, outs in TAIL_R2:
        for b0, o in zip(in0s, outs):
            for po, ho in ((0, 0), (160, 64)):
                H[:, ho + o:ho + o + 5] = np.minimum(
                    G[:, po + b0:po + b0 + 5],
                    G[:, po + b0 + delta:po + b0 + delta + 5])
    for par_ in range(2):
        for t in range(5):
            b0 = par_ * 64 + t * 11
            H[:, b0 + 5:b0 + 10] = H[:, b0:b0 + 5]
    tmp = np.empty((nb, 2, 5))
    for par_ in range(2):
        for j in range(5):
            tmp[:, par_, j] = H[:, par_ * 64 + 12 * np.arange(5) + j].min(axis=1)
    S = np.abs(x).sum(axis=1)
    got = np.empty((nb, OUT))
    for par_ in range(2):
        for j in range(5):
            got[:, 2 * j + par_] = S - tmp[:, par_, j]
    # reference: natural DP (as in test.py expected_f64)
    pl = np.maximum(-x, 0)
    pr = np.maximum(x, 0)
    cost = np.zeros((nb, 1))
    for d in range(HEIGHT):
        n0 = (1 << d) - 1
        w = 1 << d
        newc = np.empty((nb, 2 * w))
        newc[:, 0::2] = cost + pl[:, n0:n0 + w]
        newc[:, 1::2] = cost + pr[:, n0:n0 + w]
        cost = newc
    exp = np.empty((nb, OUT))
    for a in range(OUT):
        exp[:, a] = S - cost[:, a::OUT].min(axis=1)
    err = np.abs(got - exp).max()
    print("selfcheck max abs err:", err)
    assert err < 1e-9, err
    return True


if __name__ == "__main__":
    selfcheck()
